# revision 66
# baseline (speedup 1.0000x reference)
"""Multi-head attention block (QKV proj + softmax attention + out-proj +
residual + LayerNorm) on 8 TRN2 NeuronCores.

Sharding: core = (batch b, token-half g). Each core computes attention for
its 1024 query tokens over all 8 heads. K/V for the core's 4 local heads
are computed over the full 2048 tokens and exchanged with the pair partner
via AllGather; the gather latency hides under the V/Q projections.

Precision: weights are host-scaled by 8 and cast to fp8e4 (dodges fp8
subnormals; compensated exactly: exp scale /64 for Q*K, ones=8 for the
softmax denominator, LayerNorm scale-invariance with eps*64 for the
residual path). Matmuls with contraction >=256 run fp8 DoubleRow (2
contraction rows per pass); scores run bf16 (contraction = head dim 128).
The V projection runs with x as the stationary operand, producing
v[token, dh] directly (no PE transposes). The residual is injected into
the out-projection PSUM via an identity matmul. Accumulation is f32 in
PSUM; softmax statistics and LayerNorm are f32.
"""

import contextlib
import sys

if '/opt/trn_rl_repo' not in sys.path:
    sys.path.insert(0, '/opt/trn_rl_repo')

import ml_dtypes
import numpy as np

import concourse.bacc as bacc
import concourse.bass as bass
import concourse.bass_utils as bass_utils
import concourse.tile as tile
from concourse import mybir
from concourse.masks import make_identity

B, T, D, H = 4, 2048, 1024, 8
DH = 128            # head dim
TQ = T // 2         # query tokens per core
N_CORES = 8
DC = D // 128       # d-chunks of 128
KC = T // 128       # k-token chunks of 128
QC = TQ // 128      # q-token chunks of 128
EPS = 1e-5
WS = 8.0            # host-side weight scale (keeps fp8 weights normal)
SC_EXP = 1.0 / (float(np.sqrt(DH)) * WS * WS)
F32 = mybir.dt.float32
BF16 = mybir.dt.bfloat16
FP8 = mybir.dt.float8e4
AF = mybir.ActivationFunctionType
ALU = mybir.AluOpType
DR = mybir.MatmulPerfMode.DoubleRow
BF = ml_dtypes.bfloat16
E4 = ml_dtypes.float8_e4m3


def _body(nc, tc, ap, es, apply_gb):
    xt8, xq8, Wq, bq, Wk, bk, Wv, bv, Wo, gamma, beta, y = (
        ap['xt8'], ap['xq8'], ap['Wq'], ap['bq'], ap['Wk'], ap['bk'],
        ap['Wv'], ap['bv'], ap['Wo'], ap['gamma'], ap['beta'], ap['y'])

    consts = es.enter_context(tc.tile_pool(name="consts", bufs=1))
    w_pool = es.enter_context(tc.tile_pool(name="w", bufs=1))
    kt_pool = es.enter_context(tc.tile_pool(name="ktl", bufs=1))
    v_pool = es.enter_context(tc.tile_pool(name="vl", bufs=1))
    rem_pool = es.enter_context(tc.tile_pool(name="rem", bufs=1))
    qt_pool = es.enter_context(tc.tile_pool(name="qt", bufs=1))
    pt_pool = es.enter_context(tc.tile_pool(name="pt", bufs=2))
    cb_pool = es.enter_context(tc.tile_pool(name="cb", bufs=2))
    sums_pool = es.enter_context(tc.tile_pool(name="sums", bufs=2))
    y2_pool = es.enter_context(tc.tile_pool(name="y2", bufs=2))
    xq_pool = es.enter_context(tc.tile_pool(name="xq", bufs=1))
    dram = es.enter_context(tc.tile_pool(name="dram", bufs=1, space="DRAM"))

    # ---- weight / x loads (issue order = DMA priority) -------------------
    wk_t = w_pool.tile([128, DC, 512], FP8, tag="wk")
    nc.sync.dma_start(out=wk_t, in_=Wk)
    xt = w_pool.tile([128, DC, T], FP8, tag="xt")
    for i in range(DC // 2):
        nc.sync.dma_start(out=xt[:, 2 * i:2 * i + 2, :],
                          in_=xt8[:, 2 * i:2 * i + 2, :])
    wv_t = w_pool.tile([128, DC, 512], FP8, tag="wv")
    nc.sync.dma_start(out=wv_t, in_=Wv)
    wq_t = w_pool.tile([128, DC, D], FP8, tag="wq")
    nc.sync.dma_start(out=wq_t, in_=Wq)

    ident = consts.tile([128, 128], BF16, tag="ident")
    make_identity(nc, ident)
    ones = consts.tile([128, 2, 32], FP8, tag="ones")
    nc.vector.memset(ones, WS)
    eps_t = consts.tile([128, 1], F32, tag="eps")
    nc.vector.memset(eps_t, EPS * WS * WS)

    # biases arrive host-pretransposed: [128, H] / [128, 4]
    bq_t = consts.tile([128, H], F32, tag="bq")
    bk_t = consts.tile([128, 4], F32, tag="bk")
    nc.sync.dma_start(out=bq_t, in_=bq)
    nc.sync.dma_start(out=bk_t, in_=bk)

    def bcast128(name, src, n):
        t = consts.tile([128, n], F32, tag=name, name=name)
        src_b = bass.AP(tensor=src.tensor, offset=src.offset,
                        ap=[[0, 128]] + src.ap)
        nc.sync.dma_start(out=t, in_=src_b)
        return t

    bv_bc = bcast128("bv_bc", bv, 512)

    # late-phase tensors (prefetched mid-attention)
    wo_t = w_pool.tile([128, DC, D], FP8, tag="wo")
    xq_t = xq_pool.tile([128, QC, D], BF16, tag="xqs")
    ctx_all = w_pool.tile([128, QC, H, 128], FP8, tag="ctx_all")

    kt_loc = [kt_pool.tile([128, T], FP8, tag=f"ktl{j}", name=f"ktl{j}")
              for j in range(4)]
    v_loc4 = v_pool.tile([128, 4, KC, 128], FP8, tag="v4")
    kt_rem = [rem_pool.tile([128, T], FP8, tag=f"ktr{j}", name=f"ktr{j}")
              for j in range(4)]
    v_rem = [rem_pool.tile([128, KC, 128], FP8, tag=f"vr{j}", name=f"vr{j}")
             for j in range(4)]
    tmp8_pool = es.enter_context(tc.tile_pool(name="tmp8", bufs=2))
    tmpb_pool = es.enter_context(tc.tile_pool(name="tmpb", bufs=2))
    k_send = dram.tile([4, 128, T], FP8, tag="k_send")
    k_all = dram.tile([2, 4, 128, T], FP8, tag="k_all")
    v_send = dram.tile([4, 128, KC, 128], FP8, tag="v_send")
    v_all = dram.tile([2, 4, 128, KC, 128], FP8, tag="v_all")

    def fetch_k(j):
        # remote = (gathered blk0 + blk1) - local; rank-uniform, on vector
        b1 = tmp8_pool.tile([128, T], FP8, tag="b1", name="b1")
        nc.sync.dma_start(out=kt_rem[j], in_=k_all[0][j])
        nc.sync.dma_start(out=b1, in_=k_all[1][j])
        sbf = tmpb_pool.tile([128, T], BF16, tag="sbf", name="sbf")
        nc.vector.tensor_tensor(out=sbf, in0=kt_rem[j], in1=b1, op=ALU.add)
        nc.vector.tensor_tensor(out=kt_rem[j], in0=sbf, in1=kt_loc[j],
                                op=ALU.subtract)

    def fetch_v(j):
        vb1 = tmp8_pool.tile([128, KC, 128], FP8, tag="vb1", name="vb1")
        nc.sync.dma_start(out=v_rem[j], in_=v_all[0][j])
        nc.sync.dma_start(out=vb1, in_=v_all[1][j])
        vbf = tmpb_pool.tile([128, KC, 128], BF16, tag="vbf", name="vbf")
        nc.vector.tensor_tensor(out=vbf, in0=v_rem[j], in1=vb1, op=ALU.add)
        nc.vector.tensor_tensor(out=v_rem[j], in0=vbf, in1=v_loc4[:, j],
                                op=ALU.subtract)

    with contextlib.ExitStack() as es2:
        proj_ps = es2.enter_context(tc.tile_pool(name="proj_ps", bufs=4,
                                                 space="PSUM"))

        # ---- K projection, pass-major over (j, nt) groups of 8 ----------
        for grp in range(2):
            combos = [(grp * 2 + j, nt) for j in range(2)
                      for nt in range(T // 512)]
            kps = [proj_ps.tile([128, 512], F32, tag=f"psg{ci % 8}",
                                bufs=1, name="pp") for ci in range(8)]
            for i in range(DC // 2):
                for (j, nt), pp in zip(combos, kps):
                    jsl = slice(j * 128, (j + 1) * 128)
                    nsl = slice(nt * 512, (nt + 1) * 512)
                    nc.tensor.matmul(pp, wk_t[:, 2 * i:2 * i + 2, jsl],
                                     xt[:, 2 * i:2 * i + 2, nsl],
                                     start=(i == 0), stop=(i == DC // 2 - 1),
                                     perf_mode=DR)
            for (j, nt), pp in zip(combos, kps):
                nsl = slice(nt * 512, (nt + 1) * 512)
                nc.vector.tensor_scalar(out=kt_loc[j][:, nsl], in0=pp,
                                        scalar1=bk_t[:, j:j + 1],
                                        scalar2=None, op0=ALU.add)
            for j in (grp * 2, grp * 2 + 1):
                nc.sync.dma_start(out=k_send[j], in_=kt_loc[j])
        nc.gpsimd.collective_compute(
            "AllGather", ALU.bypass,
            ins=[k_send.opt()], outs=[k_all.opt()],
            replica_groups=[[0, 1], [2, 3], [4, 5], [6, 7]])

        # ---- V projection (x stationary), pass-major chunk-groups of 8 --
        for grp in range(2):
            vps = [proj_ps.tile([128, 512], F32, tag=f"psg{kc % 8}",
                                bufs=1, name="vp") for kc in range(8)]
            for i in range(DC // 2):
                for ki, vp in enumerate(vps):
                    kc = grp * 8 + ki
                    ksl = slice(kc * 128, (kc + 1) * 128)
                    nc.tensor.matmul(vp, xt[:, 2 * i:2 * i + 2, ksl],
                                     wv_t[:, 2 * i:2 * i + 2, :],
                                     start=(i == 0),
                                     stop=(i == DC // 2 - 1), perf_mode=DR)
            for ki, vp in enumerate(vps):
                kc = grp * 8 + ki
                nc.vector.tensor_tensor(
                    out=v_loc4[:, :, kc, :],
                    in0=vp.rearrange("p (a c) -> p a c", c=128),
                    in1=bv_bc.rearrange("p (a c) -> p a c", c=128),
                    op=ALU.add)
        for j in range(4):
            nc.sync.dma_start(out=v_send[j], in_=v_loc4[:, j])
        nc.gpsimd.collective_compute(
            "AllGather", ALU.bypass,
            ins=[v_send.opt()], outs=[v_all.opt()],
            replica_groups=[[0, 1], [2, 3], [4, 5], [6, 7]])

        # ---- Q projection: all 8 slots ----------------------------------
        qt = {}
        for h in range(H):
            qt[h] = qt_pool.tile([128, TQ], BF16, tag=f"qt{h}",
                                 name=f"qt{h}")
        for h in range(H):
            hsl = slice(h * 128, (h + 1) * 128)
            for nt in range(TQ // 512):
                nsl = slice(nt * 512, (nt + 1) * 512)
                qp = proj_ps.tile([128, 512], F32,
                                  tag=f"psg{(h * 2 + nt) % 8}", bufs=1,
                                  name="qp")
                for i in range(DC // 2):
                    nc.tensor.matmul(qp, wq_t[:, 2 * i:2 * i + 2, hsl],
                                     xt[:, 2 * i:2 * i + 2, nsl],
                                     start=(i == 0), stop=(i == DC // 2 - 1),
                                     perf_mode=DR)
                nc.vector.tensor_scalar(out=qt[h][:, nsl], in0=qp,
                                        scalar1=bq_t[:, h:h + 1],
                                        scalar2=None, op0=ALU.add)


    # ---- attention ------------------------------------------------------
    with contextlib.ExitStack() as es3:
        s_psum = es3.enter_context(tc.tile_pool(name="s_ps", bufs=2,
                                                space="PSUM"))
        ctx_psum = es3.enter_context(tc.tile_pool(name="ctx_ps", bufs=1,
                                                  space="PSUM"))
        sum_psum = es3.enter_context(tc.tile_pool(name="sum_ps", bufs=1,
                                                  space="PSUM"))
        def scores_exp(h, pair):
            kt_h = kt_loc[h] if h < 4 else kt_rem[h - 4]
            qt_h = qt[h]
            pt = pt_pool.tile([128, 2, TQ], FP8, tag="pt", name="pt")
            for u in range(2):
                kc = 2 * pair + u
                ksl = slice(kc * 128, (kc + 1) * 128)
                s_ps = s_psum.tile([128, TQ], F32, tag="s", name="s_ps")
                for nq in range(TQ // 512):
                    nsl = slice(nq * 512, (nq + 1) * 512)
                    nc.tensor.matmul(s_ps[:, nsl], kt_h[:, ksl],
                                     qt_h[:, nsl], start=True, stop=True)
                nc.scalar.activation(out=pt[:, u, :], in_=s_ps,
                                     func=AF.Exp, scale=SC_EXP)
            return pt

        pt_cur = scores_exp(0, 0)
        for h in range(H):
            if h < 4:
                v_h = v_loc4[:, h]
            else:
                v_h = v_rem[h - 4]
            ctx_ps = ctx_psum.tile([128, TQ], F32, tag="ctx_ps")
            sum_ps = sum_psum.tile([1, TQ], F32, tag="sum_ps")

            for pair in range(KC // 2):
                if pair + 1 < KC // 2:
                    pt_next = scores_exp(h, pair + 1)
                elif h + 1 < H:
                    pt_next = scores_exp(h + 1, 0)
                else:
                    pt_next = None
                first, last = (pair == 0), (pair == KC // 2 - 1)
                for nq in range(TQ // 512):
                    nsl = slice(nq * 512, (nq + 1) * 512)
                    nc.tensor.matmul(ctx_ps[:, nsl],
                                     v_h[:, 2 * pair:2 * pair + 2, :],
                                     pt_cur[:, :, nsl],
                                     start=first, stop=last, perf_mode=DR)
                for nq in range(TQ // 512):
                    nsl = slice(nq * 512, (nq + 1) * 512)
                    nc.tensor.matmul(sum_ps[:, nsl], ones[:, :, 0:1],
                                     pt_cur[:, :, nsl],
                                     start=first, stop=last, perf_mode=DR)
                pt_cur = pt_next

            # drain PSUM fast, normalize off the critical path
            rsum = sums_pool.tile([1, TQ], F32, tag="rsum")
            nc.vector.reciprocal_approx_fast(out=rsum, in_=sum_ps)
            ctx_bf = cb_pool.tile([128, TQ], BF16, tag="cbf")
            nc.vector.tensor_copy(out=ctx_bf, in_=ctx_ps)
            rsum_b = sums_pool.tile([128, TQ], F32, tag="rsum_b")
            nc.gpsimd.partition_broadcast(rsum_b, rsum, channels=128)
            nc.vector.tensor_tensor(
                out=ctx_all[:, :, h, :],
                in0=ctx_bf.rearrange("p (a c) -> p a c", c=128),
                in1=rsum_b.rearrange("p (a c) -> p a c", c=128),
                op=ALU.mult)
            if h == 0:
                nc.sync.dma_start(out=wo_t, in_=Wo)
                fetch_k(0)
            if h == 1:
                xq_v = bass.AP(
                    tensor=xq8.tensor, offset=xq8.offset,
                    ap=[[D, 128], [128 * D, QC], [1, D]])
                nc.sync.dma_start(out=xq_t, in_=xq_v)
                gb = [bcast128("gamma_b", gamma, D),
                      bcast128("beta_b", beta, D)] if apply_gb else None
                fetch_k(1)
                fetch_v(0)
            if h == 2:
                fetch_k(2)
                fetch_v(1)
            if h == 3:
                fetch_k(3)
                fetch_v(2)
            if h == 4:
                fetch_v(3)

    # ---- out-projection + residual + LayerNorm --------------------------
    with tc.tile_pool(name="y_ps", bufs=3, space="PSUM") as y_psum, \
            tc.tile_pool(name="ln", bufs=4) as ln_pool:
        y_tiles = {}

        def open_qc(qc):
            # inject residual + head-pairs 0-2 (no dependency on late heads)
            y_ps = y_psum.tile([128, D], F32, tag="y_ps", name=f"y{qc}")
            for no in range(D // 512):
                nsl = slice(no * 512, (no + 1) * 512)
                nc.tensor.matmul(y_ps[:, nsl], ident, xq_t[:, qc, nsl],
                                 start=True, stop=False)
            for i in range(H // 2 - 1):
                for no in range(D // 512):
                    nsl = slice(no * 512, (no + 1) * 512)
                    nc.tensor.matmul(y_ps[:, nsl],
                                     ctx_all[:, qc, 2 * i:2 * i + 2, :],
                                     wo_t[:, 2 * i:2 * i + 2, nsl],
                                     start=False, stop=False, perf_mode=DR)
            y_tiles[qc] = y_ps

        def close_qc(qc):
            qsl = slice(qc * 128, (qc + 1) * 128)
            y_ps = y_tiles.pop(qc)
            i = H // 2 - 1
            for no in range(D // 512):
                nsl = slice(no * 512, (no + 1) * 512)
                nc.tensor.matmul(y_ps[:, nsl],
                                 ctx_all[:, qc, 2 * i:2 * i + 2, :],
                                 wo_t[:, 2 * i:2 * i + 2, nsl],
                                 start=False, stop=True, perf_mode=DR)

            # drain PSUM at once so the next chunk's matmuls can start;
            # the LN chain reads the SBUF copy
            y1 = y2_pool.tile([128, D], F32, tag="y1", name=f"y1_{qc}")
            nc.vector.tensor_copy(out=y1, in_=y_ps)
            stats = ln_pool.tile([128, 2, 6], F32, tag="stats")
            nc.vector.bn_stats(out=stats[:, 0, :], in_=y1[:, 0:512])
            nc.vector.bn_stats(out=stats[:, 1, :], in_=y1[:, 512:1024])
            mv = ln_pool.tile([128, 2], F32, tag="mv")
            nc.vector.bn_aggr(out=mv, in_=stats)
            std = ln_pool.tile([128, 1], F32, tag="std")
            nc.scalar.activation(out=std, in_=mv[:, 1:2], func=AF.Sqrt,
                                 bias=eps_t)
            rstd = ln_pool.tile([128, 1], F32, tag="rstd")
            nc.vector.reciprocal(out=rstd, in_=std)
            nmr = ln_pool.tile([128, 1], F32, tag="nmr")
            nc.vector.tensor_scalar(out=nmr, in0=mv[:, 0:1], scalar1=rstd,
                                    scalar2=-1.0, op0=ALU.mult,
                                    op1=ALU.mult)
            y2 = y2_pool.tile([128, D], F32, tag="y2")
            # (y - mu) * rstd as a per-partition affine on the idle ScalarE
            nc.scalar.activation(out=y2, in_=y1, func=AF.Identity,
                                 scale=rstd, bias=nmr)
            if apply_gb:
                nc.vector.tensor_mul(out=y2, in0=y2, in1=gb[0])
                nc.vector.tensor_add(out=y2, in0=y2, in1=gb[1])
            nc.sync.dma_start(out=y[qsl, :], in_=y2)

        pend = []
        for qc in range(QC):
            open_qc(qc)
            pend.append(qc)
            if len(pend) == 2:
                close_qc(pend.pop(0))
        while pend:
            close_qc(pend.pop(0))


def build(apply_gb=True):
    nc = bacc.Bacc("TRN2", target_bir_lowering=False, debug=False,
                   enable_asserts=False, num_devices=N_CORES)
    ap = {}
    ap['xt8'] = nc.dram_tensor("xt8", [128, DC, T], FP8,
                               kind="ExternalInput").ap()
    ap['xq8'] = nc.dram_tensor("xq8", [QC, 128, D], BF16,
                               kind="ExternalInput").ap()
    ap['Wq'] = nc.dram_tensor("Wq", [128, DC, D], FP8,
                              kind="ExternalInput").ap()
    ap['bq'] = nc.dram_tensor("bq", [128, H], F32,
                              kind="ExternalInput").ap()
    ap['Wk'] = nc.dram_tensor("Wk", [128, DC, 512], FP8,
                              kind="ExternalInput").ap()
    ap['bk'] = nc.dram_tensor("bk", [128, 4], F32,
                              kind="ExternalInput").ap()
    ap['Wv'] = nc.dram_tensor("Wv", [128, DC, 512], FP8,
                              kind="ExternalInput").ap()
    ap['bv'] = nc.dram_tensor("bv", [512], F32, kind="ExternalInput").ap()
    ap['Wo'] = nc.dram_tensor("Wo", [128, DC, D], FP8,
                              kind="ExternalInput").ap()
    ap['gamma'] = nc.dram_tensor("gamma", [D], F32, kind="ExternalInput").ap()
    ap['beta'] = nc.dram_tensor("beta", [D], F32, kind="ExternalInput").ap()
    ap['y'] = nc.dram_tensor("y", [TQ, D], F32, kind="ExternalOutput").ap()

    with tile.TileContext(nc) as tc, contextlib.ExitStack() as es:
        _body(nc, tc, ap, es, apply_gb)
    nc.compile()
    return nc


def _pack_rows(w):
    """[D, N] -> [128, DC, N] with rows (c*128+p) -> [p, c]."""
    n = w.shape[1]
    return np.ascontiguousarray(
        w.reshape(DC, 128, n).transpose(1, 0, 2))


def make_in_maps(inputs):
    """Per-core input maps; x token-rotated so q tokens come first."""
    f32 = {k: np.ascontiguousarray(np.asarray(v, dtype=np.float32))
           for k, v in inputs.items()}
    # slot order per core parity g: local heads (4g..4g+3) first, so slot s
    # holds canonical head (4g+s) mod 8 -> roll Wq/bq cols & Wo rows by -4g
    gshared = []
    for gg in range(2):
        r = -4 * gg * 128
        gshared.append({
            'Wq': _pack_rows(np.roll(WS * f32['Wq'], r, axis=1)).astype(E4),
            'Wo': _pack_rows(np.roll(WS * f32['Wo'], r, axis=0)).astype(E4),
            'bq': np.ascontiguousarray(
                np.roll(WS * f32['bq'], r).reshape(H, 128).T),
            'gamma': f32['gamma'], 'beta': f32['beta'],
        })
    wk8 = WS * f32['Wk']
    wv8 = WS * f32['Wv']
    x = f32['x']
    in_maps = []
    for core in range(N_CORES):
        b, gg = divmod(core, 2)
        own = slice(512 * gg, 512 * (gg + 1))
        xr = np.roll(x[b], -TQ * gg, axis=0)
        xq8 = (WS * (xr[:TQ] + f32['bo'])).astype(BF)
        in_maps.append({
            'xt8': _pack_rows(xr.T).astype(E4),
            'xq8': np.ascontiguousarray(xq8.reshape(QC, 128, D)),
            'Wk': _pack_rows(wk8[:, own]).astype(E4),
            'bk': np.ascontiguousarray(
                (WS * f32['bk'][own]).reshape(4, 128).T),
            'Wv': _pack_rows(wv8[:, own]).astype(E4),
            'bv': WS * f32['bv'][own],
            **gshared[gg]})
    return in_maps


_NC = {}


def kernel(**inputs):
    apply_gb = not (np.all(np.asarray(inputs['gamma']) == 1.0)
                    and np.all(np.asarray(inputs['beta']) == 0.0))
    in_maps = make_in_maps(inputs)
    if apply_gb not in _NC:
        _NC[apply_gb] = build(apply_gb)
    res = bass_utils.run_bass_kernel_spmd(_NC[apply_gb], in_maps,
                                          core_ids=list(range(N_CORES)))
    out = np.empty((B, T, D), dtype=np.float32)
    for core in range(N_CORES):
        b, gg = divmod(core, 2)
        out[b, TQ * gg:TQ * (gg + 1)] = res.results[core]['y']
    return out
